# revision 13
# baseline (speedup 1.0000x reference)
"""AttentionBlock (GroupNorm + 1x1-conv QKV self-attention + residual) on 8 TRN2 cores.

Sharding: data-parallel over batch B=4 x sequence-parallel over the 4096
tokens (2 cores per batch element, each handling 2048 query rows; K/V and
GroupNorm are computed redundantly per core pair — they are cheap relative
to attention).

Per-core device kernel (attention matmuls in bf16, GN stats + residual fp32):
  - GroupNorm folded into the QKV weights: h = scale_c*x + shift_c, so q/k/v
    come straight from x with per-channel-scaled weights + effective biases.
  - q/k are computed 4x-replicated across partition strips via col-packed
    (tile_position) projection matmuls, enabling 4x row-packed S^T matmuls
    (K=32 contraction) - 4 m-blocks computed concurrently into one 4-bank
    PSUM tile, consumed by a single wide exp on the scalar engine.
  - Softmax denominator: 4x col-packed ones-matmuls accumulating over
    m-block groups; combined + reciprocal on DVE, broadcast on GpSimd.
  - P*V accumulated over m-blocks into PSUM as out_att[e, n]; output
    projection, 1/rowsum scaling, biases and residual applied at the end
    in fp32.
"""
import sys

sys.path.insert(0, "/opt/trn_rl_repo")

import numpy as np

import concourse.bass as bass
import concourse.bass_isa as bass_isa
import concourse.tile as tile
from concourse import bacc, mybir
from concourse.bass_utils import run_bass_kernel_spmd

F32 = mybir.dt.float32
BF16 = mybir.dt.bfloat16

B, C, H, W = 4, 256, 64, 64
N = H * W          # 4096 tokens
NQ = N // 2        # 2048 query rows per core
D = C // 8         # 32 qk dim
G = 32             # groups
GS = C // G        # 8 channels per group
EPS = 1e-5
P = 128            # partitions
CT = C // P        # 2 channel tiles
CH = 512           # nq chunk
NCH = NQ // CH     # 4 chunks
MB = 128           # m block
NMB = N // MB      # 32 m blocks
NG = NMB // 4      # 8 groups of 4 m-blocks per chunk
SM_SCALE = float(D) ** -0.5

_CACHE = {}
_last_in_maps = None


def _build():
    if "nc" in _CACHE:
        return _CACHE["nc"]

    nc = bacc.Bacc("TRN2", target_bir_lowering=False, debug=False, num_devices=8)

    x_ext = nc.declare_dram_parameter("x", [C, N], F32, isOutput=False)
    xq_ext = nc.declare_dram_parameter("xq", [C, NQ], F32, isOutput=False)
    wqt_ext = nc.declare_dram_parameter("wqt", [C, D], F32, isOutput=False)
    wkt_ext = nc.declare_dram_parameter("wkt", [C, D], F32, isOutput=False)
    wvt_ext = nc.declare_dram_parameter("wvt", [C, C], F32, isOutput=False)
    wpt_ext = nc.declare_dram_parameter("wpt", [C, C], F32, isOutput=False)
    bq_ext = nc.declare_dram_parameter("bq", [D, 1], F32, isOutput=False)
    bk_ext = nc.declare_dram_parameter("bk", [D, 1], F32, isOutput=False)
    bv_ext = nc.declare_dram_parameter("bv", [C, 1], F32, isOutput=False)
    bp_ext = nc.declare_dram_parameter("bp", [C, 1], F32, isOutput=False)
    gamma_ext = nc.declare_dram_parameter("gamma", [C, 1], F32, isOutput=False)
    beta_ext = nc.declare_dram_parameter("beta", [C, 1], F32, isOutput=False)
    ind16_ext = nc.declare_dram_parameter("ind16", [P, G // CT], F32, isOutput=False)
    indb_ext = nc.declare_dram_parameter("indb", [G // CT, P], F32, isOutput=False)
    mask4_ext = nc.declare_dram_parameter("mask4", [P, 1], F32, isOutput=False)
    out_ext = nc.declare_dram_parameter("out", [C, NQ], F32, isOutput=True)

    GT = G // CT  # 16 groups per channel tile
    XP = N // 4   # x DMA piece size (overlap DMA with stats)

    with tile.TileContext(nc) as tc:
        with tc.tile_pool(name="const", bufs=1) as const, \
             tc.tile_pool(name="small", bufs=1) as small:
            # ---- persistent tiles ----
            wqt_sb, wkt_sb, wvt_sb, wpt_sb = [], [], [], []
            gamma_sb, beta_sb, bv_sb, bp_sb = [], [], [], []
            for t in range(CT):
                cs = slice(t * P, (t + 1) * P)
                w1 = const.tile([P, D], F32, tag=f"wqt{t}", name=f"wqt{t}")
                nc.gpsimd.dma_start(out=w1, in_=wqt_ext[cs, :])
                wqt_sb.append(w1)
                w2 = const.tile([P, D], F32, tag=f"wkt{t}", name=f"wkt{t}")
                nc.gpsimd.dma_start(out=w2, in_=wkt_ext[cs, :])
                wkt_sb.append(w2)
                w3 = const.tile([P, C], F32, tag=f"wvt{t}", name=f"wvt{t}")
                nc.gpsimd.dma_start(out=w3, in_=wvt_ext[cs, :])
                wvt_sb.append(w3)
                w4 = const.tile([P, C], F32, tag=f"wpt{t}", name=f"wpt{t}")
                nc.gpsimd.dma_start(out=w4, in_=wpt_ext[cs, :])
                wpt_sb.append(w4)
                for lst, ext, nm in (
                    (gamma_sb, gamma_ext, "gam"),
                    (beta_sb, beta_ext, "bet"),
                    (bv_sb, bv_ext, "bv"),
                    (bp_sb, bp_ext, "bp"),
                ):
                    tl = small.tile([P, 1], F32, tag=f"{nm}{t}", name=f"{nm}{t}")
                    nc.sync.dma_start(out=tl, in_=ext[cs, :])
                    lst.append(tl)
            bq_sb = small.tile([D, 1], F32, tag="bq")
            nc.sync.dma_start(out=bq_sb, in_=bq_ext[:])
            bk_sb = small.tile([D, 1], F32, tag="bk")
            nc.sync.dma_start(out=bk_sb, in_=bk_ext[:])
            ind16_sb = small.tile([P, GT], F32, tag="ind16")
            nc.sync.dma_start(out=ind16_sb, in_=ind16_ext[:])
            indb_sb = small.tile([GT, P], F32, tag="indb")
            nc.sync.dma_start(out=indb_sb, in_=indb_ext[:])
            onec_h = small.tile([P, 1], BF16, tag="onech")
            nc.vector.memset(onec_h, 1.0)
            mask4_sb = small.tile([P, 1], F32, tag="mask4")
            nc.sync.dma_start(out=mask4_sb, in_=mask4_ext[:])
            eps_sb = small.tile([GT, 1], F32, tag="eps")
            nc.vector.memset(eps_sb, EPS)

            x_r = [const.tile([P, N], BF16, tag=f"xr{t}", name=f"xr{t}") for t in range(CT)]
            xq_r = [const.tile([P, NQ], BF16, tag=f"xqr{t}", name=f"xqr{t}") for t in range(CT)]
            xqb = [const.tile([P, NQ], F32, tag=f"xqb{t}", name=f"xqb{t}") for t in range(CT)]
            scale_sb = [small.tile([P, 1], F32, tag=f"scale{t}", name=f"scale{t}") for t in range(CT)]
            shift_sb = [small.tile([P, 1], F32, tag=f"shift{t}", name=f"shift{t}") for t in range(CT)]

            # ---- load x; GroupNorm stats overlapped with DMA ----
            with tc.tile_pool(name="ld", bufs=2) as ld, \
                 tc.tile_pool(name="gn", bufs=2) as gn, \
                 tc.tile_pool(name="gnps", bufs=1, space="PSUM") as gnps:
                xq_f = []
                for t in range(CT):
                    cs = slice(t * P, (t + 1) * P)
                    xt = ld.tile([P, N], F32, tag=f"xt{t}", name=f"xt{t}")
                    stats = gn.tile([P, 8, nc.vector.BN_STATS_DIM], F32, tag="st")
                    for pc in range(N // XP):
                        ps_ = slice(pc * XP, (pc + 1) * XP)
                        nc.sync.dma_start(out=xt[:, ps_], in_=x_ext[cs, ps_])
                        # cast on GpSimd; bf16 stats on DVE (precision is ample)
                        nc.gpsimd.tensor_copy(out=x_r[t][:, ps_], in_=xt[:, ps_])
                        for s in range(XP // 512):
                            si = pc * (XP // 512) + s
                            nc.vector.bn_stats(
                                out=stats[:, si, :],
                                in_=x_r[t][:, pc * XP + s * 512: pc * XP + (s + 1) * 512],
                            )
                    xqt = ld.tile([P, NQ], F32, tag=f"xqt{t}", name=f"xqt{t}")
                    nc.gpsimd.dma_start(out=xqt, in_=xq_ext[cs, :])
                    nc.scalar.activation(
                        out=xq_r[t], in_=xqt,
                        func=mybir.ActivationFunctionType.Copy,
                    )
                    xq_f.append(xqt)

                    mv = gn.tile([P, nc.vector.BN_AGGR_DIM], F32, tag="mv")
                    nc.vector.bn_aggr(out=mv, in_=stats)
                    mx = gn.tile([P, 2], F32, tag="mx")
                    nc.vector.tensor_copy(out=mx[:, 0:1], in_=mv[:, 0:1])
                    msq = gn.tile([P, 1], F32, tag="msq")
                    nc.vector.tensor_mul(out=msq, in0=mv[:, 0:1], in1=mv[:, 0:1])
                    nc.vector.tensor_add(out=mx[:, 1:2], in0=mv[:, 1:2], in1=msq)

                    gps = gnps.tile([GT, 2], F32, tag="gps")
                    nc.tensor.matmul(gps, ind16_sb, mx, start=True, stop=True)
                    gsb = gn.tile([GT, 2], F32, tag="gsb")
                    nc.vector.tensor_copy(out=gsb, in_=gps)
                    mg2 = gn.tile([GT, 1], F32, tag="mg2")
                    nc.vector.tensor_mul(out=mg2, in0=gsb[:, 0:1], in1=gsb[:, 0:1])
                    varg = gn.tile([GT, 1], F32, tag="varg")
                    nc.vector.tensor_sub(out=varg, in0=gsb[:, 1:2], in1=mg2)
                    sd = gn.tile([GT, 1], F32, tag="sd")
                    nc.scalar.activation(
                        out=sd, in_=varg,
                        func=mybir.ActivationFunctionType.Sqrt,
                        bias=eps_sb, scale=1.0,
                    )
                    g2 = gn.tile([GT, 2], F32, tag="g2")
                    nc.vector.tensor_copy(out=g2[:, 0:1], in_=gsb[:, 0:1])
                    nc.vector.reciprocal(out=g2[:, 1:2], in_=sd)

                    bc = gnps.tile([P, 2], F32, tag="bc")
                    nc.tensor.matmul(bc, indb_sb, g2, start=True, stop=True)
                    nc.vector.tensor_mul(out=scale_sb[t], in0=gamma_sb[t], in1=bc[:, 1:2])
                    sh1 = gn.tile([P, 1], F32, tag="sh1")
                    nc.vector.tensor_mul(out=sh1, in0=bc[:, 0:1], in1=scale_sb[t])
                    nc.vector.tensor_sub(out=shift_sb[t], in0=beta_sb[t], in1=sh1)

                # ---- scaled weights + effective biases ----
                wqt_h = [const.tile([P, D], BF16, tag=f"wqth{t}", name=f"wqth{t}") for t in range(CT)]
                wkt_h = [const.tile([P, D], BF16, tag=f"wkth{t}", name=f"wkth{t}") for t in range(CT)]
                wvt_h = [const.tile([P, C], BF16, tag=f"wvth{t}", name=f"wvth{t}") for t in range(CT)]
                wpt_h = [const.tile([P, C], BF16, tag=f"wpth{t}", name=f"wpth{t}") for t in range(CT)]
                for t in range(CT):
                    nc.vector.tensor_scalar_mul(out=wqt_h[t], in0=wqt_sb[t], scalar1=scale_sb[t])
                    nc.vector.tensor_scalar_mul(out=wkt_h[t], in0=wkt_sb[t], scalar1=scale_sb[t])
                    nc.vector.tensor_scalar_mul(out=wvt_h[t], in0=wvt_sb[t], scalar1=scale_sb[t])
                    nc.vector.tensor_copy(out=wpt_h[t], in_=wpt_sb[t])

                with tc.tile_pool(name="bps", bufs=1, space="PSUM") as bps:
                    bq_eff = small.tile([D, 1], F32, tag="bqe")
                    bk_eff = small.tile([D, 1], F32, tag="bke")
                    psq = bps.tile([D, 1], F32, tag="pq")
                    psk = bps.tile([D, 1], F32, tag="pk")
                    for t in range(CT):
                        nc.tensor.matmul(psq, wqt_sb[t], shift_sb[t], start=(t == 0), stop=(t == CT - 1))
                        nc.tensor.matmul(psk, wkt_sb[t], shift_sb[t], start=(t == 0), stop=(t == CT - 1))
                    nc.vector.tensor_add(out=bq_eff, in0=psq, in1=bq_sb)
                    nc.vector.tensor_add(out=bk_eff, in0=psk, in1=bk_sb)
                    # replicate biases across the 4 partition strips
                    bq_rep = small.tile([P, 1], F32, tag="bqrep")
                    bk_rep = small.tile([P, 1], F32, tag="bkrep")
                    for j in range(4):
                        nc.vector.tensor_copy(out=bq_rep[32 * j:32 * (j + 1), :], in_=bq_eff)
                        nc.vector.tensor_copy(out=bk_rep[32 * j:32 * (j + 1), :], in_=bk_eff)

                    bv_eff = [small.tile([P, 1], F32, tag=f"bve{e}", name=f"bve{e}") for e in range(CT)]
                    for e in range(CT):
                        ps3 = bps.tile([P, 1], F32, tag=f"pv{e}", name=f"psv{e}")
                        for t in range(CT):
                            nc.tensor.matmul(
                                ps3, wvt_sb[t][:, e * P:(e + 1) * P], shift_sb[t],
                                start=(t == 0), stop=(t == CT - 1),
                            )
                        nc.vector.tensor_add(out=bv_eff[e], in0=ps3, in1=bv_sb[e])
                    for f in range(CT):
                        ps4 = bps.tile([P, 1], F32, tag=f"pp{f}", name=f"psp{f}")
                        for e in range(CT):
                            nc.tensor.matmul(
                                ps4, wpt_sb[e][:, f * P:(f + 1) * P], bv_eff[e],
                                start=(e == 0), stop=(e == CT - 1),
                            )
                        bp_eff = small.tile([P, 1], F32, tag=f"bpe{f}", name=f"bpe{f}")
                        nc.vector.tensor_add(out=bp_eff, in0=ps4, in1=bp_sb[f])
                        nc.gpsimd.tensor_scalar_add(out=xqb[f], in0=xq_f[f], scalar1=bp_eff)

            # ---- q/k (4x partition-replicated via col-packed matmuls) + v^T ----
            q_rep = const.tile([P, NQ], BF16, tag="qrep")
            k_rep = const.tile([P, N], BF16, tag="krep")
            vt_h = const.tile([P, NMB, C], BF16, tag="vth")
            with tc.tile_pool(name="qkps", bufs=1, space="PSUM") as qkps, \
                 tc.tile_pool(name="vtps", bufs=2, space="PSUM") as vtps:
                for ch2 in range(NQ // (2 * CH)):
                    qp = qkps.tile([P, 2 * CH], F32, tag="qp")
                    for half in range(2):
                        ns = slice((2 * ch2 + half) * CH, (2 * ch2 + half + 1) * CH)
                        hs = slice(half * CH, (half + 1) * CH)
                        for t in range(CT):
                            for j in range(4):
                                nc.tensor.matmul(
                                    qp[32 * j:32 * (j + 1), hs], wqt_h[t], xq_r[t][:, ns],
                                    start=(t == 0), stop=(t == CT - 1),
                                    tile_position=(0, 32 * j),
                                )
                    ns2 = slice(2 * ch2 * CH, 2 * (ch2 + 1) * CH)
                    nc.vector.tensor_scalar_add(out=q_rep[:, ns2], in0=qp, scalar1=bq_rep)
                for ch2 in range(N // (2 * CH)):
                    kp = qkps.tile([P, 2 * CH], F32, tag="kp")
                    for half in range(2):
                        ns = slice((2 * ch2 + half) * CH, (2 * ch2 + half + 1) * CH)
                        hs = slice(half * CH, (half + 1) * CH)
                        for t in range(CT):
                            for j in range(4):
                                nc.tensor.matmul(
                                    kp[32 * j:32 * (j + 1), hs], wkt_h[t], x_r[t][:, ns],
                                    start=(t == 0), stop=(t == CT - 1),
                                    tile_position=(0, 32 * j),
                                )
                    ns2 = slice(2 * ch2 * CH, 2 * (ch2 + 1) * CH)
                    nc.vector.tensor_scalar_add(out=k_rep[:, ns2], in0=kp, scalar1=bk_rep)
                # v^T in 4-m-block granules: [128, 1024] 2-bank psum, one wide copy
                for vg in range(NMB // 4):
                    vp = vtps.tile([P, 4, C], F32, tag="vp")
                    for mloc in range(4):
                        mb = vg * 4 + mloc
                        ms = slice(mb * MB, (mb + 1) * MB)
                        for t in range(CT):
                            nc.tensor.matmul(
                                vp[:, mloc, :], x_r[t][:, ms], wvt_h[t],
                                start=(t == 0), stop=(t == CT - 1),
                            )
                    nc.scalar.activation(
                        out=vt_h[:, vg * 4:(vg + 1) * 4, :], in_=vp,
                        func=mybir.ActivationFunctionType.Copy,
                    )

            # ---- attention ----
            with tc.tile_pool(name="stps", bufs=1, space="PSUM") as stps, \
                 tc.tile_pool(name="attps", bufs=1, space="PSUM") as attps, \
                 tc.tile_pool(name="rsps", bufs=1, space="PSUM") as rsps, \
                 tc.tile_pool(name="pjps", bufs=1, space="PSUM") as pjps, \
                 tc.tile_pool(name="pp", bufs=2) as pp, \
                 tc.tile_pool(name="attsb", bufs=4) as attsb, \
                 tc.tile_pool(name="osb", bufs=4) as osb, \
                 tc.tile_pool(name="rsb", bufs=2) as rsb:
                for ch in range(NCH):
                    ns = slice(ch * CH, (ch + 1) * CH)
                    att2 = attps.tile([P, CT * CH], F32, tag="att2")
                    rs = rsps.tile([P, CH], F32, tag="rs")

                    p_tiles = [None] * NG
                    for g in range(NG + 1):
                        if g < NG:
                            # 4 row-packed S^T matmuls split across two 2-bank
                            # tiles so next group's S^T overlaps this group's exp
                            stga = stps.tile([P, 2 * CH], F32, tag="stga")
                            stgb = stps.tile([P, 2 * CH], F32, tag="stgb")
                            for j in range(4):
                                mb = g * 4 + j
                                dst = stga if j < 2 else stgb
                                nc.tensor.matmul(
                                    dst[:, (j % 2) * CH:(j % 2 + 1) * CH],
                                    k_rep[32 * j:32 * (j + 1), mb * MB:(mb + 1) * MB],
                                    q_rep[32 * j:32 * (j + 1), ns],
                                    start=True, stop=True,
                                    tile_position=(32 * j, 0),
                                )
                            pg = pp.tile([P, 4 * CH], BF16, tag="pg")
                            nc.scalar.activation(
                                out=pg[:, 0:2 * CH], in_=stga,
                                func=mybir.ActivationFunctionType.Exp,
                                scale=SM_SCALE,
                            )
                            nc.scalar.activation(
                                out=pg[:, 2 * CH:4 * CH], in_=stgb,
                                func=mybir.ActivationFunctionType.Exp,
                                scale=SM_SCALE,
                            )
                            p_tiles[g] = pg
                        if g > 0:
                            gp = g - 1
                            pg = p_tiles[gp]
                            for j in range(4):
                                nc.tensor.matmul(
                                    rs[32 * j:32 * j + 1, :],
                                    onec_h, pg[:, j * CH:(j + 1) * CH],
                                    start=(gp == 0), stop=(gp == NG - 1),
                                    tile_position=(0, 32 * j),
                                )
                            for j in range(4):
                                mb = gp * 4 + j
                                for e in range(CT):
                                    nc.tensor.matmul(
                                        att2[:, e * CH:(e + 1) * CH],
                                        vt_h[:, mb, e * P:(e + 1) * P],
                                        pg[:, j * CH:(j + 1) * CH],
                                        start=(mb == 0), stop=(mb == NMB - 1),
                                    )

                    # att->sbuf casts + projection FIRST (keeps PE fed);
                    # denominator chain runs in parallel on GpSimd/DVE
                    att_sb2 = attsb.tile([P, CT * CH], BF16, tag="attsb2")
                    nc.vector.tensor_copy(out=att_sb2, in_=att2)
                    rs_sb = rsb.tile([P, CH], F32, tag="rssb")
                    nc.vector.tensor_scalar_mul(out=rs_sb, in0=rs, scalar1=mask4_sb)
                    rsum = rsb.tile([P, CH], F32, tag="rsum")
                    nc.gpsimd.partition_all_reduce(
                        rsum, rs_sb, channels=P, reduce_op=bass_isa.ReduceOp.add,
                    )
                    pjs = []
                    for f in range(CT):
                        fs = slice(f * P, (f + 1) * P)
                        pj = pjps.tile([P, CH], F32, tag="pj", name=f"pj{f}")
                        for e in range(CT):
                            nc.tensor.matmul(
                                pj, wpt_h[e][:, fs], att_sb2[:, e * CH:(e + 1) * CH],
                                start=(e == 0), stop=(e == CT - 1),
                            )
                        # plain copy releases the pj bank without waiting on
                        # the reciprocal chain
                        pjc = osb.tile([P, CH], F32, tag="pjc", name=f"pjc{f}")
                        nc.vector.tensor_copy(out=pjc, in_=pj)
                        pjs.append(pjc)
                    rec_bc = rsb.tile([P, CH], F32, tag="recbc")
                    nc.vector.reciprocal(out=rec_bc, in_=rsum)
                    for f in range(CT):
                        fs = slice(f * P, (f + 1) * P)
                        t1 = osb.tile([P, CH], F32, tag="t1")
                        nc.vector.tensor_mul(out=t1, in0=pjs[f], in1=rec_bc)
                        o = osb.tile([P, CH], F32, tag="o")
                        nc.vector.tensor_add(out=o, in0=t1, in1=xqb[f][:, ns])
                        nc.sync.dma_start(out=out_ext[fs, ns], in_=o)

    nc.compile()
    _CACHE["nc"] = nc
    return nc


def kernel(x, gamma, beta, wq, bq, wk, bk, wv, bv, wp, bp):
    x = np.ascontiguousarray(np.asarray(x, dtype=np.float32))
    nc = _build()

    GT = G // CT
    ind16 = np.zeros((P, GT), np.float32)
    for c in range(P):
        ind16[c, c // GS] = 1.0 / GS
    indb = np.zeros((GT, P), np.float32)
    for c in range(P):
        indb[c // GS, c] = 1.0
    mask4 = np.zeros((P, 1), np.float32)
    for j in range(4):
        mask4[32 * j, 0] = 1.0

    common = {
        "wqt": np.ascontiguousarray(np.asarray(wq, np.float32).T),
        "wkt": np.ascontiguousarray(np.asarray(wk, np.float32).T),
        "wvt": np.ascontiguousarray(np.asarray(wv, np.float32).T),
        "wpt": np.ascontiguousarray(np.asarray(wp, np.float32).T),
        "bq": np.asarray(bq, np.float32).reshape(D, 1),
        "bk": np.asarray(bk, np.float32).reshape(D, 1),
        "bv": np.asarray(bv, np.float32).reshape(C, 1),
        "bp": np.asarray(bp, np.float32).reshape(C, 1),
        "gamma": np.asarray(gamma, np.float32).reshape(C, 1),
        "beta": np.asarray(beta, np.float32).reshape(C, 1),
        "ind16": ind16,
        "indb": indb,
        "mask4": mask4,
    }

    xf = x.reshape(B, C, N)
    in_maps = []
    for core in range(8):
        b, half = core // 2, core % 2
        m = dict(common)
        m["x"] = np.ascontiguousarray(xf[b])
        m["xq"] = np.ascontiguousarray(xf[b][:, half * NQ:(half + 1) * NQ])
        in_maps.append(m)

    global _last_in_maps
    _last_in_maps = in_maps
    res = run_bass_kernel_spmd(nc, in_maps, list(range(8)))

    y = np.empty((B, C, N), np.float32)
    for core in range(8):
        b, half = core // 2, core % 2
        y[b][:, half * NQ:(half + 1) * NQ] = res.results[core]["out"]
    return y.reshape(B, C, H, W)


# revision 14
# speedup vs baseline: 1.2901x; 1.2901x over previous
"""AttentionBlock (GroupNorm + 1x1-conv QKV self-attention + residual) on 8 TRN2 cores.

Sharding: data-parallel over batch B=4 x sequence-parallel over the 4096
tokens (2 cores per batch element, each handling 2048 query rows; K/V and
GroupNorm are computed redundantly per core pair — they are cheap relative
to attention).

Per-core device kernel (attention matmuls in bf16, GN stats + residual fp32):
  - GroupNorm folded into the QKV weights: h = scale_c*x + shift_c, so q/k/v
    come straight from x with per-channel-scaled weights + effective biases.
  - q/k are computed 4x-replicated across partition strips via col-packed
    (tile_position) projection matmuls, enabling 4x row-packed S^T matmuls
    (K=32 contraction) - 4 m-blocks computed concurrently into one 4-bank
    PSUM tile, consumed by a single wide exp on the scalar engine.
  - Softmax denominator: 4x col-packed ones-matmuls accumulating over
    m-block groups; combined + reciprocal on DVE, broadcast on GpSimd.
  - P*V accumulated over m-blocks into PSUM as out_att[e, n]; output
    projection, 1/rowsum scaling, biases and residual applied at the end
    in fp32.
"""
import sys

sys.path.insert(0, "/opt/trn_rl_repo")

import numpy as np

import concourse.bass as bass
import concourse.bass_isa as bass_isa
import concourse.tile as tile
from concourse import bacc, mybir
from concourse.bass_utils import run_bass_kernel_spmd

F32 = mybir.dt.float32
BF16 = mybir.dt.bfloat16

B, C, H, W = 4, 256, 64, 64
N = H * W          # 4096 tokens
NQ = N // 2        # 2048 query rows per core
D = C // 8         # 32 qk dim
G = 32             # groups
GS = C // G        # 8 channels per group
EPS = 1e-5
P = 128            # partitions
CT = C // P        # 2 channel tiles
CH = 512           # nq chunk
NCH = NQ // CH     # 4 chunks
MB = 128           # m block
NMB = N // MB      # 32 m blocks
NG = NMB // 4      # 8 groups of 4 m-blocks per chunk
SM_SCALE = float(D) ** -0.5

_CACHE = {}
_last_in_maps = None


def _build():
    if "nc" in _CACHE:
        return _CACHE["nc"]

    nc = bacc.Bacc("TRN2", target_bir_lowering=False, debug=False, num_devices=8)

    x_ext = nc.declare_dram_parameter("x", [C, N], F32, isOutput=False)
    xq_ext = nc.declare_dram_parameter("xq", [C, NQ], F32, isOutput=False)
    wqt_ext = nc.declare_dram_parameter("wqt", [C, D], F32, isOutput=False)
    wkt_ext = nc.declare_dram_parameter("wkt", [C, D], F32, isOutput=False)
    wvt_ext = nc.declare_dram_parameter("wvt", [C, C], F32, isOutput=False)
    wpt_ext = nc.declare_dram_parameter("wpt", [C, C], F32, isOutput=False)
    bq_ext = nc.declare_dram_parameter("bq", [D, 1], F32, isOutput=False)
    bk_ext = nc.declare_dram_parameter("bk", [D, 1], F32, isOutput=False)
    bv_ext = nc.declare_dram_parameter("bv", [C, 1], F32, isOutput=False)
    bp_ext = nc.declare_dram_parameter("bp", [C, 1], F32, isOutput=False)
    gamma_ext = nc.declare_dram_parameter("gamma", [C, 1], F32, isOutput=False)
    beta_ext = nc.declare_dram_parameter("beta", [C, 1], F32, isOutput=False)
    ind16_ext = nc.declare_dram_parameter("ind16", [P, G // CT], F32, isOutput=False)
    indb_ext = nc.declare_dram_parameter("indb", [G // CT, P], F32, isOutput=False)
    mask4_ext = nc.declare_dram_parameter("mask4", [P, 1], F32, isOutput=False)
    out_ext = nc.declare_dram_parameter("out", [C, NQ], F32, isOutput=True)

    GT = G // CT  # 16 groups per channel tile
    XP = N // 4   # x DMA piece size (overlap DMA with stats)

    with tile.TileContext(nc) as tc:
        with tc.tile_pool(name="const", bufs=1) as const, \
             tc.tile_pool(name="small", bufs=1) as small:
            # ---- persistent tiles ----
            wqt_sb, wkt_sb, wvt_sb, wpt_sb = [], [], [], []
            gamma_sb, beta_sb, bv_sb, bp_sb = [], [], [], []
            for t in range(CT):
                cs = slice(t * P, (t + 1) * P)
                w1 = const.tile([P, D], F32, tag=f"wqt{t}", name=f"wqt{t}")
                nc.gpsimd.dma_start(out=w1, in_=wqt_ext[cs, :])
                wqt_sb.append(w1)
                w2 = const.tile([P, D], F32, tag=f"wkt{t}", name=f"wkt{t}")
                nc.gpsimd.dma_start(out=w2, in_=wkt_ext[cs, :])
                wkt_sb.append(w2)
                w3 = const.tile([P, C], F32, tag=f"wvt{t}", name=f"wvt{t}")
                nc.gpsimd.dma_start(out=w3, in_=wvt_ext[cs, :])
                wvt_sb.append(w3)
                w4 = const.tile([P, C], F32, tag=f"wpt{t}", name=f"wpt{t}")
                nc.gpsimd.dma_start(out=w4, in_=wpt_ext[cs, :])
                wpt_sb.append(w4)
                for lst, ext, nm in (
                    (gamma_sb, gamma_ext, "gam"),
                    (beta_sb, beta_ext, "bet"),
                    (bv_sb, bv_ext, "bv"),
                    (bp_sb, bp_ext, "bp"),
                ):
                    tl = small.tile([P, 1], F32, tag=f"{nm}{t}", name=f"{nm}{t}")
                    nc.sync.dma_start(out=tl, in_=ext[cs, :])
                    lst.append(tl)
            bq_sb = small.tile([D, 1], F32, tag="bq")
            nc.sync.dma_start(out=bq_sb, in_=bq_ext[:])
            bk_sb = small.tile([D, 1], F32, tag="bk")
            nc.sync.dma_start(out=bk_sb, in_=bk_ext[:])
            ind16_sb = small.tile([P, GT], F32, tag="ind16")
            nc.sync.dma_start(out=ind16_sb, in_=ind16_ext[:])
            indb_sb = small.tile([GT, P], F32, tag="indb")
            nc.sync.dma_start(out=indb_sb, in_=indb_ext[:])
            onec_h = small.tile([P, 1], BF16, tag="onech")
            nc.vector.memset(onec_h, 1.0)
            mask4_sb = small.tile([P, 1], F32, tag="mask4")
            nc.sync.dma_start(out=mask4_sb, in_=mask4_ext[:])
            eps_sb = small.tile([GT, 1], F32, tag="eps")
            nc.vector.memset(eps_sb, EPS)

            x_r = [const.tile([P, N], BF16, tag=f"xr{t}", name=f"xr{t}") for t in range(CT)]
            xq_r = [const.tile([P, NQ], BF16, tag=f"xqr{t}", name=f"xqr{t}") for t in range(CT)]
            xqb = [const.tile([P, NQ], F32, tag=f"xqb{t}", name=f"xqb{t}") for t in range(CT)]
            scale_sb = [small.tile([P, 1], F32, tag=f"scale{t}", name=f"scale{t}") for t in range(CT)]
            shift_sb = [small.tile([P, 1], F32, tag=f"shift{t}", name=f"shift{t}") for t in range(CT)]

            # ---- load x; GroupNorm stats overlapped with DMA ----
            with tc.tile_pool(name="ld", bufs=2) as ld, \
                 tc.tile_pool(name="gn", bufs=2) as gn, \
                 tc.tile_pool(name="gnps", bufs=1, space="PSUM") as gnps:
                xq_f = []
                for t in range(CT):
                    cs = slice(t * P, (t + 1) * P)
                    xt = ld.tile([P, N], F32, tag=f"xt{t}", name=f"xt{t}")
                    stats = gn.tile([P, 8, nc.vector.BN_STATS_DIM], F32, tag="st")
                    for pc in range(N // XP):
                        ps_ = slice(pc * XP, (pc + 1) * XP)
                        nc.sync.dma_start(out=xt[:, ps_], in_=x_ext[cs, ps_])
                        nc.vector.tensor_copy(out=x_r[t][:, ps_], in_=xt[:, ps_])
                        for s in range(XP // 512):
                            si = pc * (XP // 512) + s
                            nc.vector.bn_stats(
                                out=stats[:, si, :],
                                in_=x_r[t][:, pc * XP + s * 512: pc * XP + (s + 1) * 512],
                            )
                    xqt = ld.tile([P, NQ], F32, tag=f"xqt{t}", name=f"xqt{t}")
                    nc.gpsimd.dma_start(out=xqt, in_=xq_ext[cs, :])
                    nc.scalar.activation(
                        out=xq_r[t], in_=xqt,
                        func=mybir.ActivationFunctionType.Copy,
                    )
                    xq_f.append(xqt)

                    mv = gn.tile([P, nc.vector.BN_AGGR_DIM], F32, tag="mv")
                    nc.vector.bn_aggr(out=mv, in_=stats)
                    mx = gn.tile([P, 2], F32, tag="mx")
                    nc.vector.tensor_copy(out=mx[:, 0:1], in_=mv[:, 0:1])
                    msq = gn.tile([P, 1], F32, tag="msq")
                    nc.vector.tensor_mul(out=msq, in0=mv[:, 0:1], in1=mv[:, 0:1])
                    nc.vector.tensor_add(out=mx[:, 1:2], in0=mv[:, 1:2], in1=msq)

                    gps = gnps.tile([GT, 2], F32, tag="gps")
                    nc.tensor.matmul(gps, ind16_sb, mx, start=True, stop=True)
                    gsb = gn.tile([GT, 2], F32, tag="gsb")
                    nc.vector.tensor_copy(out=gsb, in_=gps)
                    mg2 = gn.tile([GT, 1], F32, tag="mg2")
                    nc.vector.tensor_mul(out=mg2, in0=gsb[:, 0:1], in1=gsb[:, 0:1])
                    varg = gn.tile([GT, 1], F32, tag="varg")
                    nc.vector.tensor_sub(out=varg, in0=gsb[:, 1:2], in1=mg2)
                    sd = gn.tile([GT, 1], F32, tag="sd")
                    nc.scalar.activation(
                        out=sd, in_=varg,
                        func=mybir.ActivationFunctionType.Sqrt,
                        bias=eps_sb, scale=1.0,
                    )
                    g2 = gn.tile([GT, 2], F32, tag="g2")
                    nc.vector.tensor_copy(out=g2[:, 0:1], in_=gsb[:, 0:1])
                    nc.vector.reciprocal(out=g2[:, 1:2], in_=sd)

                    bc = gnps.tile([P, 2], F32, tag="bc")
                    nc.tensor.matmul(bc, indb_sb, g2, start=True, stop=True)
                    nc.vector.tensor_mul(out=scale_sb[t], in0=gamma_sb[t], in1=bc[:, 1:2])
                    sh1 = gn.tile([P, 1], F32, tag="sh1")
                    nc.vector.tensor_mul(out=sh1, in0=bc[:, 0:1], in1=scale_sb[t])
                    nc.vector.tensor_sub(out=shift_sb[t], in0=beta_sb[t], in1=sh1)

                # ---- scaled weights + effective biases ----
                wqt_h = [const.tile([P, D], BF16, tag=f"wqth{t}", name=f"wqth{t}") for t in range(CT)]
                wkt_h = [const.tile([P, D], BF16, tag=f"wkth{t}", name=f"wkth{t}") for t in range(CT)]
                wvt_h = [const.tile([P, C], BF16, tag=f"wvth{t}", name=f"wvth{t}") for t in range(CT)]
                wpt_h = [const.tile([P, C], BF16, tag=f"wpth{t}", name=f"wpth{t}") for t in range(CT)]
                for t in range(CT):
                    nc.vector.tensor_scalar_mul(out=wqt_h[t], in0=wqt_sb[t], scalar1=scale_sb[t])
                    nc.vector.tensor_scalar_mul(out=wkt_h[t], in0=wkt_sb[t], scalar1=scale_sb[t])
                    nc.vector.tensor_scalar_mul(out=wvt_h[t], in0=wvt_sb[t], scalar1=scale_sb[t])
                    nc.vector.tensor_copy(out=wpt_h[t], in_=wpt_sb[t])

                with tc.tile_pool(name="bps", bufs=1, space="PSUM") as bps:
                    bq_eff = small.tile([D, 1], F32, tag="bqe")
                    bk_eff = small.tile([D, 1], F32, tag="bke")
                    psq = bps.tile([D, 1], F32, tag="pq")
                    psk = bps.tile([D, 1], F32, tag="pk")
                    for t in range(CT):
                        nc.tensor.matmul(psq, wqt_sb[t], shift_sb[t], start=(t == 0), stop=(t == CT - 1))
                        nc.tensor.matmul(psk, wkt_sb[t], shift_sb[t], start=(t == 0), stop=(t == CT - 1))
                    nc.vector.tensor_add(out=bq_eff, in0=psq, in1=bq_sb)
                    nc.vector.tensor_add(out=bk_eff, in0=psk, in1=bk_sb)
                    # replicate biases across the 4 partition strips
                    bq_rep = small.tile([P, 1], F32, tag="bqrep")
                    bk_rep = small.tile([P, 1], F32, tag="bkrep")
                    for j in range(4):
                        nc.vector.tensor_copy(out=bq_rep[32 * j:32 * (j + 1), :], in_=bq_eff)
                        nc.vector.tensor_copy(out=bk_rep[32 * j:32 * (j + 1), :], in_=bk_eff)

                    bv_eff = [small.tile([P, 1], F32, tag=f"bve{e}", name=f"bve{e}") for e in range(CT)]
                    for e in range(CT):
                        ps3 = bps.tile([P, 1], F32, tag=f"pv{e}", name=f"psv{e}")
                        for t in range(CT):
                            nc.tensor.matmul(
                                ps3, wvt_sb[t][:, e * P:(e + 1) * P], shift_sb[t],
                                start=(t == 0), stop=(t == CT - 1),
                            )
                        nc.vector.tensor_add(out=bv_eff[e], in0=ps3, in1=bv_sb[e])
                    for f in range(CT):
                        ps4 = bps.tile([P, 1], F32, tag=f"pp{f}", name=f"psp{f}")
                        for e in range(CT):
                            nc.tensor.matmul(
                                ps4, wpt_sb[e][:, f * P:(f + 1) * P], bv_eff[e],
                                start=(e == 0), stop=(e == CT - 1),
                            )
                        bp_eff = small.tile([P, 1], F32, tag=f"bpe{f}", name=f"bpe{f}")
                        nc.vector.tensor_add(out=bp_eff, in0=ps4, in1=bp_sb[f])
                        nc.vector.tensor_scalar_add(out=xqb[f], in0=xq_f[f], scalar1=bp_eff)

            # ---- q/k (4x partition-replicated via col-packed matmuls) + v^T ----
            q_rep = const.tile([P, NQ], BF16, tag="qrep")
            k_rep = const.tile([P, N], BF16, tag="krep")
            vt_h = const.tile([P, NMB, C], BF16, tag="vth")
            with tc.tile_pool(name="qkps", bufs=1, space="PSUM") as qkps, \
                 tc.tile_pool(name="vtps", bufs=2, space="PSUM") as vtps:
                for ch2 in range(NQ // (2 * CH)):
                    qp = qkps.tile([P, 2 * CH], F32, tag="qp")
                    for half in range(2):
                        ns = slice((2 * ch2 + half) * CH, (2 * ch2 + half + 1) * CH)
                        hs = slice(half * CH, (half + 1) * CH)
                        for t in range(CT):
                            for j in range(4):
                                nc.tensor.matmul(
                                    qp[32 * j:32 * (j + 1), hs], wqt_h[t], xq_r[t][:, ns],
                                    start=(t == 0), stop=(t == CT - 1),
                                    tile_position=(0, 32 * j),
                                )
                    ns2 = slice(2 * ch2 * CH, 2 * (ch2 + 1) * CH)
                    nc.vector.tensor_scalar_add(out=q_rep[:, ns2], in0=qp, scalar1=bq_rep)
                for ch2 in range(N // (2 * CH)):
                    kp = qkps.tile([P, 2 * CH], F32, tag="kp")
                    for half in range(2):
                        ns = slice((2 * ch2 + half) * CH, (2 * ch2 + half + 1) * CH)
                        hs = slice(half * CH, (half + 1) * CH)
                        for t in range(CT):
                            for j in range(4):
                                nc.tensor.matmul(
                                    kp[32 * j:32 * (j + 1), hs], wkt_h[t], x_r[t][:, ns],
                                    start=(t == 0), stop=(t == CT - 1),
                                    tile_position=(0, 32 * j),
                                )
                    ns2 = slice(2 * ch2 * CH, 2 * (ch2 + 1) * CH)
                    nc.vector.tensor_scalar_add(out=k_rep[:, ns2], in0=kp, scalar1=bk_rep)
                # v^T in 4-m-block granules: [128, 1024] 2-bank psum, one wide copy
                for vg in range(NMB // 4):
                    vp = vtps.tile([P, 4, C], F32, tag="vp")
                    for mloc in range(4):
                        mb = vg * 4 + mloc
                        ms = slice(mb * MB, (mb + 1) * MB)
                        for t in range(CT):
                            nc.tensor.matmul(
                                vp[:, mloc, :], x_r[t][:, ms], wvt_h[t],
                                start=(t == 0), stop=(t == CT - 1),
                            )
                    nc.vector.tensor_copy(out=vt_h[:, vg * 4:(vg + 1) * 4, :], in_=vp)

            # ---- attention ----
            with tc.tile_pool(name="stps", bufs=1, space="PSUM") as stps, \
                 tc.tile_pool(name="attps", bufs=1, space="PSUM") as attps, \
                 tc.tile_pool(name="rsps", bufs=1, space="PSUM") as rsps, \
                 tc.tile_pool(name="pjps", bufs=1, space="PSUM") as pjps, \
                 tc.tile_pool(name="pp", bufs=2) as pp, \
                 tc.tile_pool(name="attsb", bufs=4) as attsb, \
                 tc.tile_pool(name="osb", bufs=4) as osb, \
                 tc.tile_pool(name="rsb", bufs=2) as rsb:
                for ch in range(NCH):
                    ns = slice(ch * CH, (ch + 1) * CH)
                    att2 = attps.tile([P, CT * CH], F32, tag="att2")
                    rs = rsps.tile([P, CH], F32, tag="rs")

                    p_tiles = [None] * NG
                    for g in range(NG + 1):
                        if g < NG:
                            # 4 row-packed S^T matmuls split across two 2-bank
                            # tiles so next group's S^T overlaps this group's exp
                            stga = stps.tile([P, 2 * CH], F32, tag="stga")
                            stgb = stps.tile([P, 2 * CH], F32, tag="stgb")
                            for j in range(4):
                                mb = g * 4 + j
                                dst = stga if j < 2 else stgb
                                nc.tensor.matmul(
                                    dst[:, (j % 2) * CH:(j % 2 + 1) * CH],
                                    k_rep[32 * j:32 * (j + 1), mb * MB:(mb + 1) * MB],
                                    q_rep[32 * j:32 * (j + 1), ns],
                                    start=True, stop=True,
                                    tile_position=(32 * j, 0),
                                )
                            pg = pp.tile([P, 4 * CH], BF16, tag="pg")
                            nc.scalar.activation(
                                out=pg[:, 0:2 * CH], in_=stga,
                                func=mybir.ActivationFunctionType.Exp,
                                scale=SM_SCALE,
                            )
                            nc.scalar.activation(
                                out=pg[:, 2 * CH:4 * CH], in_=stgb,
                                func=mybir.ActivationFunctionType.Exp,
                                scale=SM_SCALE,
                            )
                            p_tiles[g] = pg
                        if g > 0:
                            gp = g - 1
                            pg = p_tiles[gp]
                            for j in range(4):
                                nc.tensor.matmul(
                                    rs[32 * j:32 * j + 1, :],
                                    onec_h, pg[:, j * CH:(j + 1) * CH],
                                    start=(gp == 0), stop=(gp == NG - 1),
                                    tile_position=(0, 32 * j),
                                )
                            for j in range(4):
                                mb = gp * 4 + j
                                for e in range(CT):
                                    nc.tensor.matmul(
                                        att2[:, e * CH:(e + 1) * CH],
                                        vt_h[:, mb, e * P:(e + 1) * P],
                                        pg[:, j * CH:(j + 1) * CH],
                                        start=(mb == 0), stop=(mb == NMB - 1),
                                    )

                    # att->sbuf casts + projection FIRST (keeps PE fed);
                    # denominator chain runs in parallel on GpSimd/DVE
                    att_sb2 = attsb.tile([P, CT * CH], BF16, tag="attsb2")
                    nc.vector.tensor_copy(out=att_sb2, in_=att2)
                    rs_sb = rsb.tile([P, CH], F32, tag="rssb")
                    nc.vector.tensor_scalar_mul(out=rs_sb, in0=rs, scalar1=mask4_sb)
                    rsum = rsb.tile([P, CH], F32, tag="rsum")
                    nc.gpsimd.partition_all_reduce(
                        rsum, rs_sb, channels=P, reduce_op=bass_isa.ReduceOp.add,
                    )
                    pjs = []
                    for f in range(CT):
                        fs = slice(f * P, (f + 1) * P)
                        pj = pjps.tile([P, CH], F32, tag="pj", name=f"pj{f}")
                        for e in range(CT):
                            nc.tensor.matmul(
                                pj, wpt_h[e][:, fs], att_sb2[:, e * CH:(e + 1) * CH],
                                start=(e == 0), stop=(e == CT - 1),
                            )
                        # plain copy releases the pj bank without waiting on
                        # the reciprocal chain
                        pjc = osb.tile([P, CH], F32, tag="pjc", name=f"pjc{f}")
                        nc.vector.tensor_copy(out=pjc, in_=pj)
                        pjs.append(pjc)
                    rec_bc = rsb.tile([P, CH], F32, tag="recbc")
                    nc.vector.reciprocal(out=rec_bc, in_=rsum)
                    for f in range(CT):
                        fs = slice(f * P, (f + 1) * P)
                        t1 = osb.tile([P, CH], F32, tag="t1")
                        nc.vector.tensor_mul(out=t1, in0=pjs[f], in1=rec_bc)
                        o = osb.tile([P, CH], F32, tag="o")
                        nc.vector.tensor_add(out=o, in0=t1, in1=xqb[f][:, ns])
                        nc.sync.dma_start(out=out_ext[fs, ns], in_=o)

    nc.compile()
    _CACHE["nc"] = nc
    return nc


def kernel(x, gamma, beta, wq, bq, wk, bk, wv, bv, wp, bp):
    x = np.ascontiguousarray(np.asarray(x, dtype=np.float32))
    nc = _build()

    GT = G // CT
    ind16 = np.zeros((P, GT), np.float32)
    for c in range(P):
        ind16[c, c // GS] = 1.0 / GS
    indb = np.zeros((GT, P), np.float32)
    for c in range(P):
        indb[c // GS, c] = 1.0
    mask4 = np.zeros((P, 1), np.float32)
    for j in range(4):
        mask4[32 * j, 0] = 1.0

    common = {
        "wqt": np.ascontiguousarray(np.asarray(wq, np.float32).T),
        "wkt": np.ascontiguousarray(np.asarray(wk, np.float32).T),
        "wvt": np.ascontiguousarray(np.asarray(wv, np.float32).T),
        "wpt": np.ascontiguousarray(np.asarray(wp, np.float32).T),
        "bq": np.asarray(bq, np.float32).reshape(D, 1),
        "bk": np.asarray(bk, np.float32).reshape(D, 1),
        "bv": np.asarray(bv, np.float32).reshape(C, 1),
        "bp": np.asarray(bp, np.float32).reshape(C, 1),
        "gamma": np.asarray(gamma, np.float32).reshape(C, 1),
        "beta": np.asarray(beta, np.float32).reshape(C, 1),
        "ind16": ind16,
        "indb": indb,
        "mask4": mask4,
    }

    xf = x.reshape(B, C, N)
    in_maps = []
    for core in range(8):
        b, half = core // 2, core % 2
        m = dict(common)
        m["x"] = np.ascontiguousarray(xf[b])
        m["xq"] = np.ascontiguousarray(xf[b][:, half * NQ:(half + 1) * NQ])
        in_maps.append(m)

    global _last_in_maps
    _last_in_maps = in_maps
    res = run_bass_kernel_spmd(nc, in_maps, list(range(8)))

    y = np.empty((B, C, N), np.float32)
    for core in range(8):
        b, half = core // 2, core % 2
        y[b][:, half * NQ:(half + 1) * NQ] = res.results[core]["out"]
    return y.reshape(B, C, H, W)


# revision 17
# speedup vs baseline: 1.4138x; 1.0959x over previous
"""AttentionBlock (GroupNorm + 1x1-conv QKV self-attention + residual) on 8 TRN2 cores.

Sharding: data-parallel over batch B=4 x sequence-parallel over the 4096
tokens (2 cores per batch element, each handling 2048 query rows; K/V and
GroupNorm are computed redundantly per core pair — they are cheap relative
to attention).

Per-core device kernel (attention matmuls in bf16, GN stats + residual fp32):
  - GroupNorm folded into the QKV weights: h = scale_c*x + shift_c, so q/k/v
    come straight from x with per-channel-scaled weights + effective biases.
  - q/k are computed 4x-replicated across partition strips via col-packed
    (tile_position) projection matmuls, enabling 4x row-packed S^T matmuls
    (K=32 contraction) - 4 m-blocks computed concurrently into one 4-bank
    PSUM tile, consumed by a single wide exp on the scalar engine.
  - Softmax denominator: 4x col-packed ones-matmuls accumulating over
    m-block groups; combined + reciprocal on DVE, broadcast on GpSimd.
  - P*V accumulated over m-blocks into PSUM as out_att[e, n]; output
    projection, 1/rowsum scaling, biases and residual applied at the end
    in fp32.
"""
import sys

sys.path.insert(0, "/opt/trn_rl_repo")

import numpy as np

import concourse.bass as bass
import concourse.bass_isa as bass_isa
import concourse.tile as tile
from concourse.tile_rust import add_dep_helper
from concourse import bacc, mybir
from concourse.bass_utils import run_bass_kernel_spmd

F32 = mybir.dt.float32
BF16 = mybir.dt.bfloat16

B, C, H, W = 4, 256, 64, 64
N = H * W          # 4096 tokens
NQ = N // 2        # 2048 query rows per core
D = C // 8         # 32 qk dim
G = 32             # groups
GS = C // G        # 8 channels per group
EPS = 1e-5
P = 128            # partitions
CT = C // P        # 2 channel tiles
CH = 512           # nq chunk
NCH = NQ // CH     # 4 chunks
MB = 128           # m block
NMB = N // MB      # 32 m blocks
NG = NMB // 4      # 8 groups of 4 m-blocks per chunk
SM_SCALE = float(D) ** -0.5

_CACHE = {}
_last_in_maps = None


def _build():
    if "nc" in _CACHE:
        return _CACHE["nc"]

    nc = bacc.Bacc("TRN2", target_bir_lowering=False, debug=False, num_devices=8)

    x_ext = nc.declare_dram_parameter("x", [C, N], F32, isOutput=False)
    xq_ext = nc.declare_dram_parameter("xq", [C, NQ], F32, isOutput=False)
    wqt_ext = nc.declare_dram_parameter("wqt", [C, D], F32, isOutput=False)
    wkt_ext = nc.declare_dram_parameter("wkt", [C, D], F32, isOutput=False)
    wvt_ext = nc.declare_dram_parameter("wvt", [C, C], F32, isOutput=False)
    wpt_ext = nc.declare_dram_parameter("wpt", [C, C], F32, isOutput=False)
    bq_ext = nc.declare_dram_parameter("bq", [D, 1], F32, isOutput=False)
    bk_ext = nc.declare_dram_parameter("bk", [D, 1], F32, isOutput=False)
    bv_ext = nc.declare_dram_parameter("bv", [C, 1], F32, isOutput=False)
    bp_ext = nc.declare_dram_parameter("bp", [C, 1], F32, isOutput=False)
    gamma_ext = nc.declare_dram_parameter("gamma", [C, 1], F32, isOutput=False)
    beta_ext = nc.declare_dram_parameter("beta", [C, 1], F32, isOutput=False)
    ind16_ext = nc.declare_dram_parameter("ind16", [P, G // CT], F32, isOutput=False)
    indb_ext = nc.declare_dram_parameter("indb", [G // CT, P], F32, isOutput=False)
    mask4_ext = nc.declare_dram_parameter("mask4", [P, 1], F32, isOutput=False)
    out_ext = nc.declare_dram_parameter("out", [C, NQ], F32, isOutput=True)

    GT = G // CT  # 16 groups per channel tile
    XP = N // 4   # x DMA piece size (overlap DMA with stats)

    with tile.TileContext(nc) as tc:
        with tc.tile_pool(name="const", bufs=1) as const, \
             tc.tile_pool(name="small", bufs=1) as small:
            # ---- persistent tiles ----
            wqt_sb, wkt_sb, wvt_sb, wpt_sb = [], [], [], []
            gamma_sb, beta_sb, bv_sb, bp_sb = [], [], [], []
            for t in range(CT):
                cs = slice(t * P, (t + 1) * P)
                w1 = const.tile([P, D], F32, tag=f"wqt{t}", name=f"wqt{t}")
                nc.gpsimd.dma_start(out=w1, in_=wqt_ext[cs, :])
                wqt_sb.append(w1)
                w2 = const.tile([P, D], F32, tag=f"wkt{t}", name=f"wkt{t}")
                nc.gpsimd.dma_start(out=w2, in_=wkt_ext[cs, :])
                wkt_sb.append(w2)
                w3 = const.tile([P, C], F32, tag=f"wvt{t}", name=f"wvt{t}")
                nc.gpsimd.dma_start(out=w3, in_=wvt_ext[cs, :])
                wvt_sb.append(w3)
                w4 = const.tile([P, C], F32, tag=f"wpt{t}", name=f"wpt{t}")
                nc.gpsimd.dma_start(out=w4, in_=wpt_ext[cs, :])
                wpt_sb.append(w4)
                for lst, ext, nm in (
                    (gamma_sb, gamma_ext, "gam"),
                    (beta_sb, beta_ext, "bet"),
                    (bv_sb, bv_ext, "bv"),
                    (bp_sb, bp_ext, "bp"),
                ):
                    tl = small.tile([P, 1], F32, tag=f"{nm}{t}", name=f"{nm}{t}")
                    nc.sync.dma_start(out=tl, in_=ext[cs, :])
                    lst.append(tl)
            bq_sb = small.tile([D, 1], F32, tag="bq")
            nc.sync.dma_start(out=bq_sb, in_=bq_ext[:])
            bk_sb = small.tile([D, 1], F32, tag="bk")
            nc.sync.dma_start(out=bk_sb, in_=bk_ext[:])
            ind16_sb = small.tile([P, GT], F32, tag="ind16")
            nc.sync.dma_start(out=ind16_sb, in_=ind16_ext[:])
            indb_sb = small.tile([GT, P], F32, tag="indb")
            nc.sync.dma_start(out=indb_sb, in_=indb_ext[:])
            onec_h = small.tile([P, 1], BF16, tag="onech")
            nc.vector.memset(onec_h, 1.0)
            mask4_sb = small.tile([P, 1], F32, tag="mask4")
            nc.sync.dma_start(out=mask4_sb, in_=mask4_ext[:])
            eps_sb = small.tile([GT, 1], F32, tag="eps")
            nc.vector.memset(eps_sb, EPS)

            x_r = [const.tile([P, N], BF16, tag=f"xr{t}", name=f"xr{t}") for t in range(CT)]
            xq_r = [const.tile([P, NQ], BF16, tag=f"xqr{t}", name=f"xqr{t}") for t in range(CT)]
            xqb = [const.tile([P, NQ], F32, tag=f"xqb{t}", name=f"xqb{t}") for t in range(CT)]
            scale_sb = [small.tile([P, 1], F32, tag=f"scale{t}", name=f"scale{t}") for t in range(CT)]
            shift_sb = [small.tile([P, 1], F32, tag=f"shift{t}", name=f"shift{t}") for t in range(CT)]

            # ---- load x; GroupNorm stats overlapped with DMA ----
            with tc.tile_pool(name="ld", bufs=2) as ld, \
                 tc.tile_pool(name="gn", bufs=2) as gn, \
                 tc.tile_pool(name="gnps", bufs=1, space="PSUM") as gnps:
                xq_f = []
                for t in range(CT):
                    cs = slice(t * P, (t + 1) * P)
                    xt = ld.tile([P, N], F32, tag=f"xt{t}", name=f"xt{t}")
                    stats = gn.tile([P, 8, nc.vector.BN_STATS_DIM], F32, tag="st")
                    for pc in range(N // XP):
                        ps_ = slice(pc * XP, (pc + 1) * XP)
                        nc.sync.dma_start(out=xt[:, ps_], in_=x_ext[cs, ps_])
                        nc.vector.tensor_copy(out=x_r[t][:, ps_], in_=xt[:, ps_])
                        for s in range(XP // 512):
                            si = pc * (XP // 512) + s
                            nc.vector.bn_stats(
                                out=stats[:, si, :],
                                in_=x_r[t][:, pc * XP + s * 512: pc * XP + (s + 1) * 512],
                            )
                    xqt = ld.tile([P, NQ], F32, tag=f"xqt{t}", name=f"xqt{t}")
                    nc.gpsimd.dma_start(out=xqt, in_=xq_ext[cs, :])
                    nc.scalar.activation(
                        out=xq_r[t], in_=xqt,
                        func=mybir.ActivationFunctionType.Copy,
                    )
                    xq_f.append(xqt)

                    mv = gn.tile([P, nc.vector.BN_AGGR_DIM], F32, tag="mv")
                    nc.vector.bn_aggr(out=mv, in_=stats)
                    mx = gn.tile([P, 2], F32, tag="mx")
                    nc.vector.tensor_copy(out=mx[:, 0:1], in_=mv[:, 0:1])
                    msq = gn.tile([P, 1], F32, tag="msq")
                    nc.vector.tensor_mul(out=msq, in0=mv[:, 0:1], in1=mv[:, 0:1])
                    nc.vector.tensor_add(out=mx[:, 1:2], in0=mv[:, 1:2], in1=msq)

                    gps = gnps.tile([GT, 2], F32, tag="gps")
                    nc.tensor.matmul(gps, ind16_sb, mx, start=True, stop=True)
                    gsb = gn.tile([GT, 2], F32, tag="gsb")
                    nc.vector.tensor_copy(out=gsb, in_=gps)
                    mg2 = gn.tile([GT, 1], F32, tag="mg2")
                    nc.vector.tensor_mul(out=mg2, in0=gsb[:, 0:1], in1=gsb[:, 0:1])
                    varg = gn.tile([GT, 1], F32, tag="varg")
                    nc.vector.tensor_sub(out=varg, in0=gsb[:, 1:2], in1=mg2)
                    sd = gn.tile([GT, 1], F32, tag="sd")
                    nc.scalar.activation(
                        out=sd, in_=varg,
                        func=mybir.ActivationFunctionType.Sqrt,
                        bias=eps_sb, scale=1.0,
                    )
                    g2 = gn.tile([GT, 2], F32, tag="g2")
                    nc.vector.tensor_copy(out=g2[:, 0:1], in_=gsb[:, 0:1])
                    nc.vector.reciprocal(out=g2[:, 1:2], in_=sd)

                    bc = gnps.tile([P, 2], F32, tag="bc")
                    nc.tensor.matmul(bc, indb_sb, g2, start=True, stop=True)
                    nc.vector.tensor_mul(out=scale_sb[t], in0=gamma_sb[t], in1=bc[:, 1:2])
                    sh1 = gn.tile([P, 1], F32, tag="sh1")
                    nc.vector.tensor_mul(out=sh1, in0=bc[:, 0:1], in1=scale_sb[t])
                    nc.vector.tensor_sub(out=shift_sb[t], in0=beta_sb[t], in1=sh1)

                # ---- scaled weights + effective biases ----
                wqt_h = [const.tile([P, D], BF16, tag=f"wqth{t}", name=f"wqth{t}") for t in range(CT)]
                wkt_h = [const.tile([P, D], BF16, tag=f"wkth{t}", name=f"wkth{t}") for t in range(CT)]
                wvt_h = [const.tile([P, C], BF16, tag=f"wvth{t}", name=f"wvth{t}") for t in range(CT)]
                wpt_h = [const.tile([P, C], BF16, tag=f"wpth{t}", name=f"wpth{t}") for t in range(CT)]
                for t in range(CT):
                    nc.vector.tensor_scalar_mul(out=wqt_h[t], in0=wqt_sb[t], scalar1=scale_sb[t])
                    nc.vector.tensor_scalar_mul(out=wkt_h[t], in0=wkt_sb[t], scalar1=scale_sb[t])
                    nc.vector.tensor_scalar_mul(out=wvt_h[t], in0=wvt_sb[t], scalar1=scale_sb[t])
                    nc.vector.tensor_copy(out=wpt_h[t], in_=wpt_sb[t])

                with tc.tile_pool(name="bps", bufs=1, space="PSUM") as bps:
                    bq_eff = small.tile([D, 1], F32, tag="bqe")
                    bk_eff = small.tile([D, 1], F32, tag="bke")
                    psq = bps.tile([D, 1], F32, tag="pq")
                    psk = bps.tile([D, 1], F32, tag="pk")
                    for t in range(CT):
                        nc.tensor.matmul(psq, wqt_sb[t], shift_sb[t], start=(t == 0), stop=(t == CT - 1))
                        nc.tensor.matmul(psk, wkt_sb[t], shift_sb[t], start=(t == 0), stop=(t == CT - 1))
                    nc.vector.tensor_add(out=bq_eff, in0=psq, in1=bq_sb)
                    nc.vector.tensor_add(out=bk_eff, in0=psk, in1=bk_sb)
                    # replicate biases across the 2 partition strips
                    bq_rep = small.tile([64, 1], F32, tag="bqrep")
                    bk_rep = small.tile([64, 1], F32, tag="bkrep")
                    for j in range(2):
                        nc.vector.tensor_copy(out=bq_rep[32 * j:32 * (j + 1), :], in_=bq_eff)
                        nc.vector.tensor_copy(out=bk_rep[32 * j:32 * (j + 1), :], in_=bk_eff)

                    bv_eff = [small.tile([P, 1], F32, tag=f"bve{e}", name=f"bve{e}") for e in range(CT)]
                    for e in range(CT):
                        ps3 = bps.tile([P, 1], F32, tag=f"pv{e}", name=f"psv{e}")
                        for t in range(CT):
                            nc.tensor.matmul(
                                ps3, wvt_sb[t][:, e * P:(e + 1) * P], shift_sb[t],
                                start=(t == 0), stop=(t == CT - 1),
                            )
                        nc.vector.tensor_add(out=bv_eff[e], in0=ps3, in1=bv_sb[e])
                    for f in range(CT):
                        ps4 = bps.tile([P, 1], F32, tag=f"pp{f}", name=f"psp{f}")
                        for e in range(CT):
                            nc.tensor.matmul(
                                ps4, wpt_sb[e][:, f * P:(f + 1) * P], bv_eff[e],
                                start=(e == 0), stop=(e == CT - 1),
                            )
                        bp_eff = small.tile([P, 1], F32, tag=f"bpe{f}", name=f"bpe{f}")
                        nc.vector.tensor_add(out=bp_eff, in0=ps4, in1=bp_sb[f])
                        nc.vector.tensor_scalar_add(out=xqb[f], in0=xq_f[f], scalar1=bp_eff)

            # ---- q/k (4x partition-replicated via col-packed matmuls) + v^T ----
            q_rep = const.tile([64, NQ], BF16, tag="qrep")
            k_rep = const.tile([64, N], BF16, tag="krep")
            vt_h = const.tile([P, NMB, C], BF16, tag="vth")
            with tc.tile_pool(name="qkps", bufs=1, space="PSUM") as qkps, \
                 tc.tile_pool(name="vtps", bufs=2, space="PSUM") as vtps:
                for ch2 in range(NQ // (2 * CH)):
                    qp = qkps.tile([64, 2 * CH], F32, tag="qp")
                    for half in range(2):
                        ns = slice((2 * ch2 + half) * CH, (2 * ch2 + half + 1) * CH)
                        hs = slice(half * CH, (half + 1) * CH)
                        for t in range(CT):
                            for j in range(2):
                                nc.tensor.matmul(
                                    qp[32 * j:32 * (j + 1), hs], wqt_h[t], xq_r[t][:, ns],
                                    start=(t == 0), stop=(t == CT - 1),
                                    tile_position=(0, 32 * j),
                                )
                    ns2 = slice(2 * ch2 * CH, 2 * (ch2 + 1) * CH)
                    nc.vector.tensor_scalar_add(out=q_rep[:, ns2], in0=qp, scalar1=bq_rep)
                for ch2 in range(N // (2 * CH)):
                    kp = qkps.tile([64, 2 * CH], F32, tag="kp")
                    for half in range(2):
                        ns = slice((2 * ch2 + half) * CH, (2 * ch2 + half + 1) * CH)
                        hs = slice(half * CH, (half + 1) * CH)
                        for t in range(CT):
                            for j in range(2):
                                nc.tensor.matmul(
                                    kp[32 * j:32 * (j + 1), hs], wkt_h[t], x_r[t][:, ns],
                                    start=(t == 0), stop=(t == CT - 1),
                                    tile_position=(0, 32 * j),
                                )
                    ns2 = slice(2 * ch2 * CH, 2 * (ch2 + 1) * CH)
                    nc.vector.tensor_scalar_add(out=k_rep[:, ns2], in0=kp, scalar1=bk_rep)
                # v^T in 4-m-block granules: [128, 1024] 2-bank psum, one wide copy
                for vg in range(NMB // 4):
                    vp = vtps.tile([P, 4, C], F32, tag="vp")
                    for mloc in range(4):
                        mb = vg * 4 + mloc
                        ms = slice(mb * MB, (mb + 1) * MB)
                        for t in range(CT):
                            nc.tensor.matmul(
                                vp[:, mloc, :], x_r[t][:, ms], wvt_h[t],
                                start=(t == 0), stop=(t == CT - 1),
                            )
                    nc.vector.tensor_copy(out=vt_h[:, vg * 4:(vg + 1) * 4, :], in_=vp)

            # ---- attention ----
            with tc.tile_pool(name="stps", bufs=2, space="PSUM") as stps, \
                 tc.tile_pool(name="attps", bufs=1, space="PSUM") as attps, \
                 tc.tile_pool(name="rsps", bufs=1, space="PSUM") as rsps, \
                 tc.tile_pool(name="pp", bufs=3) as pp, \
                 tc.tile_pool(name="attsb", bufs=4) as attsb, \
                 tc.tile_pool(name="osb", bufs=4) as osb, \
                 tc.tile_pool(name="rsb", bufs=2) as rsb:
                for ch in range(NCH):
                    ns = slice(ch * CH, (ch + 1) * CH)
                    att2 = attps.tile([P, CT * CH], F32, tag="att2")
                    rs = rsps.tile([P, CH], F32, tag="rs")

                    NG2 = NMB // 2
                    p_tiles = [None] * NG2
                    for g in range(NG2 + 1):
                        if g < NG2:
                            # 2 row-packed S^T matmuls into a double-buffered
                            # 2-bank tile; exp overlaps the next group's S^T
                            stg = stps.tile([P, 2 * CH], F32, tag="stg")
                            for j in range(2):
                                mb = g * 2 + j
                                nc.tensor.matmul(
                                    stg[:, j * CH:(j + 1) * CH],
                                    k_rep[32 * j:32 * (j + 1), mb * MB:(mb + 1) * MB],
                                    q_rep[32 * j:32 * (j + 1), ns],
                                    start=True, stop=True,
                                    tile_position=(32 * j, 0),
                                )
                            pg = pp.tile([P, 2 * CH], BF16, tag="pg")
                            nc.scalar.activation(
                                out=pg, in_=stg,
                                func=mybir.ActivationFunctionType.Exp,
                                scale=SM_SCALE,
                            )
                            p_tiles[g] = pg
                        if g > 0:
                            gp = g - 1
                            pg = p_tiles[gp]
                            for j in range(2):
                                nc.tensor.matmul(
                                    rs[32 * j:32 * j + 1, :],
                                    onec_h, pg[:, j * CH:(j + 1) * CH],
                                    start=(gp == 0), stop=(gp == NG2 - 1),
                                    tile_position=(0, 32 * j),
                                )
                            for j in range(2):
                                mb = gp * 2 + j
                                for e in range(CT):
                                    nc.tensor.matmul(
                                        att2[:, e * CH:(e + 1) * CH],
                                        vt_h[:, mb, e * P:(e + 1) * P],
                                        pg[:, j * CH:(j + 1) * CH],
                                        start=(mb == 0), stop=(mb == NMB - 1),
                                    )

                    # att->sbuf cast + projection first (keeps PE fed);
                    # denominator runs on GpSimd/DVE in parallel and is applied
                    # as a divide after the projection
                    att_sb2 = attsb.tile([P, CT * CH], BF16, tag="attsb2")
                    nc.vector.tensor_copy(out=att_sb2, in_=att2)
                    rs_sb = rsb.tile([P, CH], F32, tag="rssb")
                    nc.vector.tensor_scalar_mul(out=rs_sb, in0=rs, scalar1=mask4_sb)
                    rsum = rsb.tile([P, CH], F32, tag="rsum")
                    nc.gpsimd.partition_all_reduce(
                        rsum, rs_sb, channels=P, reduce_op=bass_isa.ReduceOp.add,
                    )
                    pjs = []
                    for f in range(CT):
                        fs = slice(f * P, (f + 1) * P)
                        pj = rsps.tile([P, CH], F32, tag="pj", name=f"pj{f}")
                        for e in range(CT):
                            nc.tensor.matmul(
                                pj, wpt_h[e][:, fs], att_sb2[:, e * CH:(e + 1) * CH],
                                start=(e == 0), stop=(e == CT - 1),
                            )
                        # plain copy releases the pj bank without waiting on
                        # the denominator
                        pjc = osb.tile([P, CH], F32, tag="pjc", name=f"pjc{f}")
                        pjc_inst = nc.vector.tensor_copy(out=pjc, in_=pj)
                        pjs.append(pjc)
                    rec_bc = rsb.tile([P, CH], F32, tag="recbc")
                    rec_inst = nc.vector.reciprocal(out=rec_bc, in_=rsum)
                    # keep the reciprocal behind the pj copies in the DVE
                    # stream so it never blocks the att cast -> proj path
                    add_dep_helper(rec_inst.ins, pjc_inst.ins, sync=False,
                                   reason="recip after pj copies")
                    for f in range(CT):
                        fs = slice(f * P, (f + 1) * P)
                        t1 = osb.tile([P, CH], F32, tag="t1")
                        nc.vector.tensor_mul(out=t1, in0=pjs[f], in1=rec_bc)
                        o = osb.tile([P, CH], F32, tag="o")
                        nc.vector.tensor_add(out=o, in0=t1, in1=xqb[f][:, ns])
                        nc.sync.dma_start(out=out_ext[fs, ns], in_=o)

    nc.compile()
    _CACHE["nc"] = nc
    return nc


def kernel(x, gamma, beta, wq, bq, wk, bk, wv, bv, wp, bp):
    x = np.ascontiguousarray(np.asarray(x, dtype=np.float32))
    nc = _build()

    GT = G // CT
    ind16 = np.zeros((P, GT), np.float32)
    for c in range(P):
        ind16[c, c // GS] = 1.0 / GS
    indb = np.zeros((GT, P), np.float32)
    for c in range(P):
        indb[c // GS, c] = 1.0
    mask4 = np.zeros((P, 1), np.float32)
    for j in range(2):
        mask4[32 * j, 0] = 1.0

    common = {
        "wqt": np.ascontiguousarray(np.asarray(wq, np.float32).T),
        "wkt": np.ascontiguousarray(np.asarray(wk, np.float32).T),
        "wvt": np.ascontiguousarray(np.asarray(wv, np.float32).T),
        "wpt": np.ascontiguousarray(np.asarray(wp, np.float32).T),
        "bq": np.asarray(bq, np.float32).reshape(D, 1),
        "bk": np.asarray(bk, np.float32).reshape(D, 1),
        "bv": np.asarray(bv, np.float32).reshape(C, 1),
        "bp": np.asarray(bp, np.float32).reshape(C, 1),
        "gamma": np.asarray(gamma, np.float32).reshape(C, 1),
        "beta": np.asarray(beta, np.float32).reshape(C, 1),
        "ind16": ind16,
        "indb": indb,
        "mask4": mask4,
    }

    xf = x.reshape(B, C, N)
    in_maps = []
    for core in range(8):
        b, half = core // 2, core % 2
        m = dict(common)
        m["x"] = np.ascontiguousarray(xf[b])
        m["xq"] = np.ascontiguousarray(xf[b][:, half * NQ:(half + 1) * NQ])
        in_maps.append(m)

    global _last_in_maps
    _last_in_maps = in_maps
    res = run_bass_kernel_spmd(nc, in_maps, list(range(8)))

    y = np.empty((B, C, N), np.float32)
    for core in range(8):
        b, half = core // 2, core % 2
        y[b][:, half * NQ:(half + 1) * NQ] = res.results[core]["out"]
    return y.reshape(B, C, H, W)
